# revision 2
# baseline (speedup 1.0000x reference)
"""Causal multi-head self-attention (B=2, S=2048, D=1024, H=16) on 8 trn2 cores.

Sharding: tensor-parallel over heads — core c owns heads (2c, 2c+1), both
batches, full sequence. Per core: QKV projections for its 2 heads, RoPE,
causal attention, output-projection partial product. Each core writes its
partial y (bf16) straight to DRAM; the host completes the 8-way reduction
while unsharding. No cross-core communication — cores are fully independent,
so no core ever stalls on a peer.

All matmuls run as float32r (TF32-like, 1 cyc/row at N>=256), fp32 PSUM
accumulation. Softmax skips max-subtraction (scores ~ N(0,1), exp < 300).
Rowsums come free from an appended ones-column on V. Causal masking is done
post-exp with gpsimd affine_select on the diagonal blocks only.
"""
import math
import numpy as np

import concourse.bass as bass
from concourse import bacc
import concourse.mybir as mybir
from concourse.tile import TileContext
from concourse.bass_utils import run_bass_kernel_spmd

THETA = 10000.0
B, S, D, H = 2, 2048, 1024, 16
DH = D // H          # 64
NC = 8               # cores
HPC = H // NC        # heads per core = 2
R = B * S            # 4096 flat rows
SCALE = 1.0 / math.sqrt(DH)

f32 = mybir.dt.float32
f32r = mybir.dt.float32r
bf16 = mybir.dt.bfloat16

_CACHE = {}


def _build(_DBG_REPS=1):
    nc = bacc.Bacc(num_devices=NC)

    xt = nc.declare_dram_parameter("xt", [D, R], f32r, isOutput=False)
    wq = nc.declare_dram_parameter("wq", [D, 2 * DH], f32r, isOutput=False)
    wk = nc.declare_dram_parameter("wk", [D, 2 * DH], f32r, isOutput=False)
    wv = nc.declare_dram_parameter("wv", [D, 2 * DH], f32r, isOutput=False)
    wo = nc.declare_dram_parameter("wo", [2 * DH, D], f32r, isOutput=False)
    cost = nc.declare_dram_parameter("cost", [128, S], f32, isOutput=False)
    sint = nc.declare_dram_parameter("sint", [128, S], f32, isOutput=False)
    ones = nc.declare_dram_parameter("ones", [128, DH], f32r, isOutput=False)
    ident = nc.declare_dram_parameter("ident", [128, 128], f32, isOutput=False)
    yo = nc.declare_dram_parameter("yo", [R, D], bf16, isOutput=True)

    NQ = 4            # xT column quarters in phase A
    QW = R // NQ      # 1024 rows per quarter
    NB = QW // 512    # 2 proj psum blocks per quarter

    with TileContext(nc) as tc:
        import contextlib
        ctx = contextlib.ExitStack()
        with ctx:
            # ---- persistent pools (whole kernel) ----
            pers = ctx.enter_context(tc.tile_pool(name="pers", bufs=1))
            exp_pool = ctx.enter_context(tc.tile_pool(name="expp", bufs=4))
            aux = ctx.enter_context(tc.tile_pool(name="aux", bufs=2))

            q_rope = pers.tile([128, R], f32r, name="q_rope")
            k_rope = pers.tile([128, R], f32r, name="k_rope")
            # V with ones column, natural rows layout: per batch [128, 16*130]
            v_sb = [pers.tile([128, (S // 128) * 130], f32r, name=f"v_sb{b}") for b in range(B)]
            attn = [pers.tile([128, S], f32r, name=f"attn{b}") for b in range(B)]
            wo_sb = pers.tile([128, D], f32r, name="wo_sb")
            ones_sb = pers.tile([128, DH], f32r, name="ones_sb")
            nc.sync.dma_start(out=wo_sb[:, :], in_=wo[:, :])
            nc.sync.dma_start(out=ones_sb[:, :], in_=ones[:, :])
            id_sb = pers.tile([128, 128], f32, name="id_sb")
            nc.sync.dma_start(out=id_sb[:, :], in_=ident[:, :])

            # ---- phase A: projections + RoPE + V assembly ----
            for _rep in range(_DBG_REPS):
              with tc.tile_pool(name="phA", bufs=1) as pha, \
                   tc.tile_pool(name="xtp", bufs=8) as xtp, \
                   tc.tile_pool(name="ropetmp", bufs=1) as rtp, \
                   tc.tile_pool(name="prj_ps", bufs=6, space="PSUM") as prj, \
                   tc.tile_pool(name="tp_ps", bufs=2, space="PSUM") as tpp:

                  wq_sb = pha.tile([128, 8, 2 * DH], f32r, name="wq_sb")
                  wk_sb = pha.tile([128, 8, 2 * DH], f32r, name="wk_sb")
                  wv_sb = pha.tile([128, 8, 2 * DH], f32r, name="wv_sb")
                  nc.sync.dma_start(out=wq_sb[:, :, :], in_=wq.rearrange("(t p) m -> p t m", p=128))
                  nc.sync.dma_start(out=wk_sb[:, :, :], in_=wk.rearrange("(t p) m -> p t m", p=128))
                  nc.sync.dma_start(out=wv_sb[:, :, :], in_=wv.rearrange("(t p) m -> p t m", p=128))
                  cos_sb = pha.tile([128, S], f32, name="cos_sb")
                  sin_sb = pha.tile([128, S], f32, name="sin_sb")
                  nc.sync.dma_start(out=cos_sb[:, :], in_=cost[:, :])
                  nc.sync.dma_start(out=sin_sb[:, :], in_=sint[:, :])

                  for qr in range(NQ):
                      c0 = qr * QW           # global row offset of this quarter
                      bq = c0 // S           # batch of this quarter
                      s0 = c0 % S            # seq offset of this quarter
                      xts = []
                      for k in range(8):
                          xk = xtp.tile([128, QW], f32r, name=f"xt{qr}_{k}", tag="xt")
                          nc.sync.dma_start(out=xk[:, :], in_=xt[k * 128:(k + 1) * 128, c0:c0 + QW])
                          xts.append(xk)

                      for tname, wsb, rope in (("q", wq_sb, q_rope), ("k", wk_sb, k_rope)):
                          pss = []
                          for n in range(NB):
                              ps = prj.tile([128, 512], f32, name=f"p{tname}{qr}{n}", tag="prj")
                              for k in range(8):
                                  nc.tensor.matmul(ps[:, :], wsb[:, k, :], xts[k][:, n * 512:(n + 1) * 512],
                                                   start=(k == 0), stop=(k == 7))
                              pss.append(ps)
                          # RoPE: partitions [0:64]=evens (h0e|h1e), [64:128]=odds
                          raw = rtp.tile([128, QW], f32, name=f"raw{tname}{qr}", tag="raw")
                          olo = rtp.tile([64, QW], f32, name=f"olo{tname}{qr}", tag="olo")
                          ehi = rtp.tile([128, QW], f32, name=f"ehi{tname}{qr}", tag="ehi")
                          p1 = rtp.tile([64, QW], f32, name=f"p1{tname}{qr}", tag="p1")
                          p3 = rtp.tile([128, QW], f32, name=f"p3{tname}{qr}", tag="p3")
                          for n in range(NB):
                              cs = slice(n * 512, (n + 1) * 512)
                              gs = slice(s0 + n * 512, s0 + (n + 1) * 512)
                              nc.scalar.copy(raw[:, cs], pss[n][:, :])
                              nc.vector.tensor_tensor(p1[0:64, cs], pss[n][0:64, :], cos_sb[0:64, gs], mybir.AluOpType.mult)
                              nc.vector.tensor_tensor(p3[64:128, cs], pss[n][64:128, :], cos_sb[64:128, gs], mybir.AluOpType.mult)
                          nc.sync.dma_start(out=olo[0:64, :], in_=raw[64:128, :])
                          nc.sync.dma_start(out=ehi[64:128, :], in_=raw[0:64, :])
                          gq = slice(s0, s0 + QW)
                          t2 = rtp.tile([64, QW], f32, name=f"t2{tname}{qr}", tag="t2")
                          t4 = rtp.tile([128, QW], f32, name=f"t4{tname}{qr}", tag="t4")
                          nc.vector.tensor_tensor(t2[0:64, :], olo[0:64, :], sin_sb[0:64, gq], mybir.AluOpType.mult)
                          nc.vector.tensor_tensor(t4[64:128, :], ehi[64:128, :], sin_sb[64:128, gq], mybir.AluOpType.mult)
                          ero = rtp.tile([64, QW], f32r, name=f"ero{tname}{qr}", tag="ero")
                          oro = rtp.tile([128, QW], f32r, name=f"oro{tname}{qr}", tag="oro")
                          nc.vector.tensor_tensor(ero[0:64, :], p1[0:64, :], t2[0:64, :], mybir.AluOpType.subtract)
                          nc.vector.tensor_tensor(oro[64:128, :], p3[64:128, :], t4[64:128, :], mybir.AluOpType.add)
                          # shuffle to head-contiguous: [h0e|h0o|h1e|h1o]
                          nc.sync.dma_start(out=rope[0:32, c0:c0 + QW], in_=ero[0:32, :])
                          nc.sync.dma_start(out=rope[64:96, c0:c0 + QW], in_=ero[32:64, :])
                          nc.sync.dma_start(out=rope[32:64, c0:c0 + QW], in_=oro[64:96, :])
                          nc.sync.dma_start(out=rope[96:128, c0:c0 + QW], in_=oro[96:128, :])

                      # V: transposed projection (N=512) then PE-transpose to natural
                      vt_sb = rtp.tile([128, QW], f32, name=f"vt{qr}", tag="vt")
                      for n in range(NB):
                          ps = prj.tile([128, 512], f32, name=f"pv{qr}{n}", tag="prj")
                          for k in range(8):
                              nc.tensor.matmul(ps[:, :], wv_sb[:, k, :], xts[k][:, n * 512:(n + 1) * 512],
                                               start=(k == 0), stop=(k == 7))
                          nc.vector.tensor_copy(vt_sb[:, n * 512:(n + 1) * 512], ps[:, :])
                      for rt in range(QW // 128):
                          gr = c0 + rt * 128                    # global row
                          sk = (gr % S) // 128                  # key tile within batch
                          vb = v_sb[gr // S]
                          tp = tpp.tile([128, 128], f32, name=f"tp{qr}{rt}", tag="tp")
                          nc.tensor.transpose(tp[:, :], vt_sb[:, rt * 128:(rt + 1) * 128], id_sb[:, :])
                          dst = vb[:, sk * 130: sk * 130 + 130].rearrange("p (h c) -> p h c", c=65)
                          src = tp[:, :].rearrange("p (h c) -> p h c", c=64)
                          nc.vector.tensor_copy(dst[:, :, 0:64], src[:, :, :])
                          nc.sync.dma_start(out=dst[:, :, 64:65],
                                            in_=ones_sb[:, 0:2].rearrange("p (h c) -> p h c", c=1))

              # ---- attention + output projection ----
              rctx = contextlib.ExitStack()
              sc_ps = rctx.enter_context(tc.tile_pool(name=f"sc_ps{_rep}", bufs=2, space="PSUM"))
              pv_ps = rctx.enter_context(tc.tile_pool(name=f"pv_ps{_rep}", bufs=2, space="PSUM"))
              ax_ps = rctx.enter_context(tc.tile_pool(name=f"ax_ps{_rep}", bufs=2, space="PSUM"))
              for b in range(B):
                  for qh in range(2):              # row-chunk of 1024 (4 q-blocks)
                      cc = b * 2 + qh
                      for h in range(HPC):
                          for qp in range(2):      # qb pair
                              qbs = (qh * 4 + qp * 2, qh * 4 + qp * 2 + 1)
                              pv = pv_ps.tile([65, 512], f32, name=f"pv{_rep}{cc}{h}{qp}", tag="pv")
                              for qi, qb in enumerate(qbs):
                                  nsk = 2 * (qb + 1)
                                  q_sl = slice(b * S + qb * 256, b * S + (qb + 1) * 256)
                                  for ch0 in range(0, nsk, 4):
                                      m = min(4, nsk - ch0)
                                      sc = sc_ps.tile([128, 1024], f32, name=f"sc{_rep}{cc}{h}{qp}{qi}{ch0}", tag="sc")
                                      for j in range(m):
                                          sk = ch0 + j
                                          k_sl = slice(b * S + sk * 128, b * S + (sk + 1) * 128)
                                          o = slice(j * 256, (j + 1) * 256)
                                          nc.tensor.matmul(sc[:, o], k_rope[64 * h:64 * h + 64, k_sl],
                                                           q_rope[64 * h:64 * h + 64, q_sl],
                                                           start=True, stop=True)
                                      ex = exp_pool.tile([128, 1024], f32r, name=f"ex{_rep}{cc}{h}{qp}{qi}{ch0}", tag="ex")
                                      nc.scalar.activation(ex[:, 0:m * 256], sc[:, 0:m * 256],
                                                           mybir.ActivationFunctionType.Exp, scale=SCALE)
                                      for j in range(m):
                                          sk = ch0 + j
                                          o = slice(j * 256, (j + 1) * 256)
                                          if sk == 2 * qb:      # diagonal masking
                                              nc.gpsimd.affine_select(ex[:, o], ex[:, o], [[1, 256]],
                                                                      mybir.AluOpType.is_ge, 0.0,
                                                                      base=0, channel_multiplier=-1)
                                          elif sk == 2 * qb + 1:
                                              nc.gpsimd.affine_select(ex[:, o], ex[:, o], [[1, 256]],
                                                                      mybir.AluOpType.is_ge, 0.0,
                                                                      base=-128, channel_multiplier=-1)
                                      for j in range(m):
                                          sk = ch0 + j
                                          o = slice(j * 256, (j + 1) * 256)
                                          nc.tensor.matmul(pv[:, qi * 256:(qi + 1) * 256],
                                                           v_sb[b][:, sk * 130 + 65 * h: sk * 130 + 65 * h + 65],
                                                           ex[:, o],
                                                           start=(sk == 0), stop=(sk == nsk - 1))
                              # normalize: out = pv[0:64] * (1/rowsum broadcast)
                              rec = aux.tile([1, 512], f32r, name=f"rec{_rep}{cc}{h}{qp}", tag="rec")
                              with nc.allow_low_precision(reason="softmax reciprocal"):
                                  nc.vector.reciprocal(rec[0:1, :], pv[64:65, :])
                              bc = aux.tile([64, 512], f32r, name=f"bc{_rep}{cc}{h}{qp}", tag="bc")
                              bcp = ax_ps.tile([128, 512], f32, name=f"bcp{_rep}{cc}{h}{qp}", tag="axp")
                              nc.tensor.matmul(bcp[0:64, :], ones_sb[0:1, 0:64], rec[0:1, :],
                                               start=True, stop=True)
                              nc.vector.tensor_copy(bc[0:64, :], bcp[0:64, :])
                              a_sl = slice((qh * 2 + qp) * 512, (qh * 2 + qp + 1) * 512)
                              if h == 0:
                                  nc.vector.tensor_tensor(attn[b][0:64, a_sl], pv[0:64, :], bc[0:64, :],
                                                          mybir.AluOpType.mult)
                              else:
                                  hs = aux.tile([64, 512], f32r, name=f"hs{_rep}{cc}{qp}", tag="hs")
                                  nc.vector.tensor_tensor(hs[0:64, :], pv[0:64, :], bc[0:64, :],
                                                          mybir.AluOpType.mult)
                                  nc.sync.dma_start(out=attn[b][64:128, a_sl], in_=hs[0:64, :])
                      # y chunk: rows b*S + qh*1024 .. +1024 — bf16 partials to DRAM
                      for rt in range(8):
                          gr = b * S + qh * 1024 + rt * 128
                          for nb2 in range(2):
                              yp = ax_ps.tile([128, 512], f32, name=f"yp{_rep}{cc}{rt}{nb2}", tag="axp")
                              nc.tensor.matmul(yp[:, :], attn[b][:, (gr % S):(gr % S) + 128],
                                               wo_sb[:, nb2 * 512:(nb2 + 1) * 512],
                                               start=True, stop=True)
                              yc = aux.tile([128, 512], bf16, name=f"yc{_rep}{cc}{rt}{nb2}", tag="yc", bufs=3)
                              if (rt + nb2) % 2 == 0:
                                  nc.scalar.copy(yc[:, :], yp[:, :])
                              else:
                                  nc.vector.tensor_copy(yc[:, :], yp[:, :])
                              nc.sync.dma_start(out=yo[gr:gr + 128, nb2 * 512:(nb2 + 1) * 512],
                                                in_=yc[:, :])
              rctx.close()
    nc.finalize()
    return nc


def _host_inputs(in_features, token_positions, Wq, Wk, Wv, Wo):
    x = np.ascontiguousarray(in_features, dtype=np.float32).reshape(R, D)
    xt = np.ascontiguousarray(x.T)
    pos = np.asarray(token_positions, dtype=np.float64)
    inv = THETA ** (-np.arange(0, DH, 2, dtype=np.float64) / DH)   # [32]
    ang = pos[None, :] * inv[:, None]                              # [32, S]
    cos32 = np.cos(ang).astype(np.float32)
    sin32 = np.sin(ang).astype(np.float32)
    cost = np.tile(cos32, (4, 1))                                  # [128, S]
    sint = np.tile(sin32, (4, 1))
    ones = np.ones((128, DH), dtype=np.float32)
    ident = np.eye(128, dtype=np.float32)

    in_maps = []
    for c in range(NC):
        h0 = HPC * c
        rows = []
        for j in range(HPC):
            rows += [(h0 + j) * DH + 2 * i for i in range(DH // 2)]      # evens
        for j in range(HPC):
            rows += [(h0 + j) * DH + 2 * i + 1 for i in range(DH // 2)]  # odds
        wq_c = np.ascontiguousarray(Wq[rows, :].T, dtype=np.float32)     # [D, 128]
        wk_c = np.ascontiguousarray(Wk[rows, :].T, dtype=np.float32)
        vrows = list(range(h0 * DH, (h0 + HPC) * DH))
        wv_c = np.ascontiguousarray(Wv[vrows, :].T, dtype=np.float32)    # [D, 128]
        wo_c = np.ascontiguousarray(Wo[:, vrows].T, dtype=np.float32)    # [128, D]
        in_maps.append({
            "xt": xt, "wq": wq_c, "wk": wk_c, "wv": wv_c, "wo": wo_c,
            "cost": cost, "sint": sint, "ones": ones, "ident": ident,
        })
    return in_maps


def _assemble(results):
    acc = np.zeros((R, D), dtype=np.float32)
    for c in range(NC):
        acc += np.asarray(results[c]["yo"], dtype=np.float32)
    return acc.reshape(B, S, D)


def kernel(in_features, token_positions, Wq, Wk, Wv, Wo):
    if "nc" not in _CACHE:
        _CACHE["nc"] = _build()
    nc = _CACHE["nc"]
    in_maps = _host_inputs(in_features, token_positions, Wq, Wk, Wv, Wo)
    res = run_bass_kernel_spmd(nc, in_maps, list(range(NC)))
    return _assemble(res.results)


# revision 13
# speedup vs baseline: 1.2629x; 1.2629x over previous
"""Causal multi-head self-attention (B=2, S=2048, D=1024, H=16) on 8 trn2 cores.

Sharding: tensor-parallel over heads — core c owns heads (2c, 2c+1), both
batches, full sequence. Per core: QKV projections for its 2 heads, RoPE,
causal attention, output-projection partial product. Each core writes its
partial y (bf16) straight to DRAM; the host completes the 8-way reduction
while unsharding (a concat+sum over disjoint head groups). There is NO
cross-core communication: every core's instruction stream is independent, so
no core ever stalls on a peer's launch skew or a collective rendezvous —
that coupling dominated the previous ReduceScatter design's measured time.

Precision: x/Wq/Wk/Wv and q/k (post-RoPE) are bf16 (1 cyc/row on the PE at
any N); V, exp(scores) and Wo are f32r; PSUM accumulates fp32. Partial-y
outputs are bf16 (the 8-way host sum keeps total rel err ~3e-3 vs the 2e-2
gate). Softmax skips max-subtraction (scores ~ N(0,1), exp < 300). Rowsums
come free from an appended ones-column on V (memset once, V layout
[v(64)|one] per head per 128-key tile). Causal masking is post-exp via
gpsimd affine_select on diagonal blocks only.

RoPE is 3 DVE ops per [128 x QW] tile: per-head partition layout
[even(32) | odd(32)], the host sin table carries the rotation sign (-sin on
even rows, +sin on odd rows), so out = ps*cos + swap32(ps)*sin_eff with one
partition-swapped copy (2 sb->sb DMAs), writing q_rope/k_rope in place.

DMA count is minimized (HWDGE overhead ~600ns/op): xt streams in half-
quarter [128,4,1024] tiles double-buffered, y drains in [128,4,1024] tiles,
quarter-0 xt is issued directly after Wq so the first matmul starts ~5us in.
"""
import math
import numpy as np

import concourse.bass as bass
from concourse import bacc
import concourse.mybir as mybir
from concourse.tile import TileContext
from concourse.bass_utils import run_bass_kernel_spmd

THETA = 10000.0
B, S, D, H = 2, 2048, 1024, 16
DH = D // H          # 64
NC = 8               # cores
HPC = H // NC        # heads per core = 2
R = B * S            # 4096 flat rows
SCALE = 1.0 / math.sqrt(DH)

f32 = mybir.dt.float32
f32r = mybir.dt.float32r
bf16 = mybir.dt.bfloat16

_CACHE = {}


def _build(_DBG_REPS=1):
    nc = bacc.Bacc(num_devices=NC)

    xt = nc.declare_dram_parameter("xt", [D, R], bf16, isOutput=False)
    wq = nc.declare_dram_parameter("wq", [D, 2 * DH], bf16, isOutput=False)
    wk = nc.declare_dram_parameter("wk", [D, 2 * DH], bf16, isOutput=False)
    wv = nc.declare_dram_parameter("wv", [D, 2 * DH], bf16, isOutput=False)
    wo = nc.declare_dram_parameter("wo", [2 * DH, D], f32r, isOutput=False)
    cost = nc.declare_dram_parameter("cost", [128, S], f32, isOutput=False)
    sint = nc.declare_dram_parameter("sint", [128, S], f32, isOutput=False)
    ones = nc.declare_dram_parameter("ones", [128, DH], f32r, isOutput=False)
    ident = nc.declare_dram_parameter("ident", [128, 128], f32, isOutput=False)
    yo = nc.declare_dram_parameter("yo", [R, D], bf16, isOutput=True)

    NQ = 4            # xT column quarters in phase A
    QW = R // NQ      # 1024 rows per quarter
    NB = QW // 512    # 2 proj psum blocks per quarter
    KT = S // 128     # 16 key tiles per batch

    with TileContext(nc) as tc:
        import contextlib
        ctx = contextlib.ExitStack()
        with ctx:
            # ---- persistent pools (whole kernel) ----
            pers = ctx.enter_context(tc.tile_pool(name="pers", bufs=1))
            exp_pool = ctx.enter_context(tc.tile_pool(name="expp", bufs=4))
            aux = ctx.enter_context(tc.tile_pool(name="aux", bufs=2))

            q_rope = pers.tile([128, R], bf16, name="q_rope")
            k_rope = pers.tile([128, R], bf16, name="k_rope")
            # V per batch, natural rows layout; per key tile 130 cols:
            #   per head: [v(64) | one]
            v_sb = [pers.tile([128, KT * 130], f32r, name=f"v_sb{b}") for b in range(B)]
            attn = [pers.tile([128, S], f32r, name=f"attn{b}") for b in range(B)]
            wo_sb = pers.tile([128, D], f32r, name="wo_sb")
            ones_sb = pers.tile([128, DH], f32r, name="ones_sb")
            id_sb = pers.tile([128, 128], f32, name="id_sb")
            for b in range(B):
                nc.vector.memset(
                    v_sb[b][:, :].bitcast(f32).rearrange("p (t c) -> p t c", c=65)[:, :, 64:65], 1.0)

            # ---- phase A: projections + RoPE + V assembly ----
            for _rep in range(_DBG_REPS):
              with tc.tile_pool(name="phA", bufs=1) as pha, \
                   tc.tile_pool(name="xtp", bufs=2) as xtp, \
                   tc.tile_pool(name="ropetmp", bufs=2) as rtp, \
                   tc.tile_pool(name="prj_ps", bufs=6, space="PSUM") as prj, \
                   tc.tile_pool(name="tp_ps", bufs=2, space="PSUM") as tpp:

                  wq_sb = pha.tile([128, 8, 2 * DH], bf16, name="wq_sb")
                  wk_sb = pha.tile([128, 8, 2 * DH], bf16, name="wk_sb")
                  wv_sb = pha.tile([128, 8, 2 * DH], bf16, name="wv_sb")
                  cos_sb = pha.tile([128, S], f32, name="cos_sb")
                  sin_sb = pha.tile([128, S], f32, name="sin_sb")
                  nc.sync.dma_start(out=wq_sb[:, :, :], in_=wq.rearrange("(t p) m -> p t m", p=128))
                  xtb0 = [xtp.tile([128, 4, QW], bf16, name=f"xt0{hh}", tag=f"xt{hh}")
                          for hh in range(2)]
                  for hh in range(2):
                      nc.sync.dma_start(
                          out=xtb0[hh][:, :, :],
                          in_=xt[hh * 512:(hh + 1) * 512, 0:QW].rearrange(
                              "(t p) m -> p t m", p=128))
                  nc.sync.dma_start(out=wk_sb[:, :, :], in_=wk.rearrange("(t p) m -> p t m", p=128))
                  nc.sync.dma_start(out=wv_sb[:, :, :], in_=wv.rearrange("(t p) m -> p t m", p=128))
                  nc.sync.dma_start(out=cos_sb[:, :], in_=cost[:, :])
                  nc.sync.dma_start(out=sin_sb[:, :], in_=sint[:, :])
                  if _rep == 0:
                      nc.sync.dma_start(out=id_sb[:, :], in_=ident[:, :])
                      nc.sync.dma_start(out=ones_sb[:, :], in_=ones[:, :])
                      nc.sync.dma_start(out=wo_sb[:, :], in_=wo[:, :])

                  for qr in range(NQ):
                      c0 = qr * QW           # global row offset of this quarter
                      s0 = c0 % S            # seq offset of this quarter
                      if qr == 0:
                          xtb = xtb0
                      else:
                          xtb = [xtp.tile([128, 4, QW], bf16, name=f"xt{qr}{hh}", tag=f"xt{hh}")
                                 for hh in range(2)]
                          for hh in range(2):
                              nc.sync.dma_start(
                                  out=xtb[hh][:, :, :],
                                  in_=xt[hh * 512:(hh + 1) * 512, c0:c0 + QW].rearrange(
                                      "(t p) m -> p t m", p=128))
                      xts = [xtb[k // 4][:, k % 4, :] for k in range(8)]

                      for tname, wsb, rope in (("q", wq_sb, q_rope), ("k", wk_sb, k_rope)):
                          pss = []
                          for n in range(NB):
                              ps = prj.tile([128, 512], f32, name=f"p{tname}{qr}{n}", tag="prj")
                              for k in range(8):
                                  nc.tensor.matmul(ps[:, :], wsb[:, k, :], xts[k][:, n * 512:(n + 1) * 512],
                                                   start=(k == 0), stop=(k == 7))
                              pss.append(ps)
                          # RoPE: per head rows [even(32) | odd(32)]; sin table
                          # carries the sign, so out = ps*cos + swap32(ps)*sin.
                          raw = rtp.tile([128, QW], f32, name=f"raw{tname}", tag="raw")
                          pcs = rtp.tile([128, QW], f32, name=f"pcs{tname}", tag="pcs")
                          for n in range(NB):
                              cs = slice(n * 512, (n + 1) * 512)
                              gs = slice(s0 + n * 512, s0 + (n + 1) * 512)
                              nc.scalar.copy(raw[:, cs], pss[n][:, :])
                              nc.vector.tensor_tensor(pcs[:, cs], pss[n][:, :], cos_sb[:, gs], mybir.AluOpType.mult)
                          swp = rtp.tile([128, QW], f32, name=f"swp{tname}", tag="swp")
                          for r0 in range(0, 128, 64):
                              nc.sync.dma_start(out=swp[r0:r0 + 32, :], in_=raw[r0 + 32:r0 + 64, :])
                              nc.sync.dma_start(out=swp[r0 + 32:r0 + 64, :], in_=raw[r0:r0 + 32, :])
                          gq = slice(s0, s0 + QW)
                          tsn = rtp.tile([128, QW], f32, name=f"tsn{tname}", tag="tsn")
                          nc.vector.tensor_tensor(tsn[:, :], swp[:, :], sin_sb[:, gq], mybir.AluOpType.mult)
                          nc.vector.tensor_tensor(rope[:, c0:c0 + QW], pcs[:, :], tsn[:, :], mybir.AluOpType.add)

                      # V: transposed projection (N=512) then PE-transpose to natural
                      vt_sb = rtp.tile([128, QW], f32, name=f"vt{qr}", tag="vt")
                      for n in range(NB):
                          ps = prj.tile([128, 512], f32, name=f"pv{qr}{n}", tag="prj")
                          for k in range(8):
                              nc.tensor.matmul(ps[:, :], wv_sb[:, k, :], xts[k][:, n * 512:(n + 1) * 512],
                                               start=(k == 0), stop=(k == 7))
                          nc.vector.tensor_copy(vt_sb[:, n * 512:(n + 1) * 512], ps[:, :])
                      for rt in range(QW // 128):
                          gr = c0 + rt * 128                    # global row
                          sk = (gr % S) // 128                  # key tile within batch
                          vb = v_sb[gr // S]
                          tp = tpp.tile([128, 128], f32, name=f"tp{qr}{rt}", tag="tp")
                          nc.tensor.transpose(tp[:, :], vt_sb[:, rt * 128:(rt + 1) * 128], id_sb[:, :])
                          dst = vb[:, sk * 130: sk * 130 + 130]
                          nc.vector.tensor_copy(dst[:, 0:64], tp[:, 0:64])
                          nc.vector.tensor_copy(dst[:, 65:129], tp[:, 64:128])

              # ---- attention + output projection ----
              rctx = contextlib.ExitStack()
              sc_ps = rctx.enter_context(tc.tile_pool(name=f"sc_ps{_rep}", bufs=2, space="PSUM"))
              pv_ps = rctx.enter_context(tc.tile_pool(name=f"pv_ps{_rep}", bufs=2, space="PSUM"))
              ax_ps = rctx.enter_context(tc.tile_pool(name=f"ax_ps{_rep}", bufs=2, space="PSUM"))
              ysb_pool = rctx.enter_context(tc.tile_pool(name=f"ysb{_rep}", bufs=2))
              for b in range(B):
                  for qh in range(2):              # row-chunk of 1024 (4 q-blocks)
                      cc = b * 2 + qh
                      for h in range(HPC):
                          hs_cc = None
                          if h == 1:
                              hs_cc = aux.tile([64, 1024], f32r, name=f"hs{_rep}{cc}", tag="hs")
                          for qp in range(2):      # qb pair
                              qbs = (qh * 4 + qp * 2, qh * 4 + qp * 2 + 1)
                              pvt = pv_ps.tile([65, 512], f32, name=f"pv{_rep}{cc}{h}{qp}", tag="pv")
                              pv = pvt[0:65, :]
                              for qi, qb in enumerate(qbs):
                                  nsk = 2 * (qb + 1)
                                  q_sl = slice(b * S + qb * 256, b * S + (qb + 1) * 256)
                                  for ch0 in range(0, nsk, 4):
                                      m = min(4, nsk - ch0)
                                      sc = sc_ps.tile([128, 1024], f32, name=f"sc{_rep}{cc}{h}{qp}{qi}{ch0}", tag="sc")
                                      for j in range(m):
                                          sk = ch0 + j
                                          k_sl = slice(b * S + sk * 128, b * S + (sk + 1) * 128)
                                          o = slice(j * 256, (j + 1) * 256)
                                          nc.tensor.matmul(sc[:, o], k_rope[64 * h:64 * h + 64, k_sl],
                                                           q_rope[64 * h:64 * h + 64, q_sl],
                                                           start=True, stop=True)
                                      ex = exp_pool.tile([128, 1024], f32r, name=f"ex{_rep}{cc}{h}{qp}{qi}{ch0}", tag="ex")
                                      nc.scalar.activation(ex[:, 0:m * 256], sc[:, 0:m * 256],
                                                           mybir.ActivationFunctionType.Exp, scale=SCALE)
                                      for j in range(m):
                                          sk = ch0 + j
                                          o = slice(j * 256, (j + 1) * 256)
                                          if sk == 2 * qb:      # diagonal masking
                                              nc.gpsimd.affine_select(ex[:, o], ex[:, o], [[1, 256]],
                                                                      mybir.AluOpType.is_ge, 0.0,
                                                                      base=0, channel_multiplier=-1)
                                          elif sk == 2 * qb + 1:
                                              nc.gpsimd.affine_select(ex[:, o], ex[:, o], [[1, 256]],
                                                                      mybir.AluOpType.is_ge, 0.0,
                                                                      base=-128, channel_multiplier=-1)
                                      for j in range(m):
                                          sk = ch0 + j
                                          o = slice(j * 256, (j + 1) * 256)
                                          nc.tensor.matmul(pv[:, qi * 256:(qi + 1) * 256],
                                                           v_sb[b][:, sk * 130 + 65 * h: sk * 130 + 65 * h + 65],
                                                           ex[:, o],
                                                           start=(sk == 0), stop=(sk == nsk - 1))
                              # normalize: attn rows = pv_vals * (1/rowsum broadcast)
                              rec = aux.tile([1, 512], f32r, name=f"rec{_rep}{cc}{h}{qp}", tag="rec")
                              with nc.allow_low_precision(reason="softmax reciprocal"):
                                  nc.vector.reciprocal(rec[0:1, :], pv[64:65, :])
                              bc = aux.tile([64, 512], f32r, name=f"bc{_rep}{cc}{h}{qp}", tag="bc")
                              bcp = ax_ps.tile([64, 512], f32, name=f"bcp{_rep}{cc}{h}{qp}", tag="axp")
                              nc.tensor.matmul(bcp[0:64, :], ones_sb[0:1, 0:64], rec[0:1, :],
                                               start=True, stop=True)
                              nc.vector.tensor_copy(bc[0:64, :], bcp[0:64, :])
                              a_sl = slice((qh * 2 + qp) * 512, (qh * 2 + qp + 1) * 512)
                              if h == 0:
                                  nc.vector.tensor_tensor(attn[b][0:64, a_sl], pv[0:64, :], bc[0:64, :],
                                                          mybir.AluOpType.mult)
                              else:
                                  nc.vector.tensor_tensor(hs_cc[0:64, qp * 512:(qp + 1) * 512],
                                                          pv[0:64, :], bc[0:64, :],
                                                          mybir.AluOpType.mult)
                          if h == 1:
                              nc.sync.dma_start(out=attn[b][64:128, qh * 1024:(qh + 1) * 1024],
                                                in_=hs_cc[0:64, :])
                      # y chunk: rows b*S + qh*1024 .. +1024 — bf16 partials, one DMA
                      y_sb = ysb_pool.tile([128, 8, D], bf16, name=f"ysb{_rep}{cc}", tag="ysb")
                      for rt in range(8):
                          gr = b * S + qh * 1024 + rt * 128
                          for nb2 in range(2):
                              yp = ax_ps.tile([128, 512], f32, name=f"yp{_rep}{cc}{rt}{nb2}", tag="axp")
                              nc.tensor.matmul(yp[:, :], attn[b][:, (gr % S):(gr % S) + 128],
                                               wo_sb[:, nb2 * 512:(nb2 + 1) * 512],
                                               start=True, stop=True)
                              if (rt + nb2) % 2 == 0:
                                  nc.scalar.copy(y_sb[:, rt, nb2 * 512:(nb2 + 1) * 512], yp[:, :])
                              else:
                                  nc.vector.tensor_copy(y_sb[:, rt, nb2 * 512:(nb2 + 1) * 512], yp[:, :])
                      r0 = b * S + qh * 1024
                      for hh in range(2):
                          rr = r0 + hh * 512
                          nc.sync.dma_start(
                              out=yo[rr:rr + 512, :].rearrange("(t p) m -> p t m", p=128),
                              in_=y_sb[:, hh * 4:(hh + 1) * 4, :])
              rctx.close()
    nc.finalize()
    return nc


def _host_inputs(in_features, token_positions, Wq, Wk, Wv, Wo):
    import ml_dtypes
    bf = ml_dtypes.bfloat16
    x = np.ascontiguousarray(in_features, dtype=np.float32).reshape(R, D)
    xt = np.ascontiguousarray(x.T.astype(bf))
    pos = np.asarray(token_positions, dtype=np.float64)
    inv = THETA ** (-np.arange(0, DH, 2, dtype=np.float64) / DH)   # [32]
    ang = pos[None, :] * inv[:, None]                              # [32, S]
    cos32 = np.cos(ang).astype(np.float32)
    sin32 = np.sin(ang).astype(np.float32)
    # rows per head-half: [even(32) | odd(32)] x 2 heads; sin sign folded in
    cost = np.concatenate([cos32, cos32, cos32, cos32], 0)         # [128, S]
    sint = np.concatenate([-sin32, sin32, -sin32, sin32], 0)
    ones = np.ones((128, DH), dtype=np.float32)
    ident = np.eye(128, dtype=np.float32)

    in_maps = []
    for c in range(NC):
        h0 = HPC * c
        rows = []
        for j in range(HPC):
            rows += [(h0 + j) * DH + 2 * i for i in range(DH // 2)]      # evens
            rows += [(h0 + j) * DH + 2 * i + 1 for i in range(DH // 2)]  # odds
        wq_c = np.ascontiguousarray(Wq[rows, :].T.astype(bf))            # [D, 128]
        wk_c = np.ascontiguousarray(Wk[rows, :].T.astype(bf))
        vrows = list(range(h0 * DH, (h0 + HPC) * DH))
        wv_c = np.ascontiguousarray(Wv[vrows, :].T.astype(bf))           # [D, 128]
        wo_c = np.ascontiguousarray(Wo[:, vrows].T, dtype=np.float32)    # [128, D]
        in_maps.append({
            "xt": xt, "wq": wq_c, "wk": wk_c, "wv": wv_c, "wo": wo_c,
            "cost": cost, "sint": sint, "ones": ones, "ident": ident,
        })
    return in_maps


def _assemble(results):
    acc = np.zeros((R, D), dtype=np.float32)
    for c in range(NC):
        acc += np.asarray(results[c]["yo"], dtype=np.float32)
    return acc.reshape(B, S, D)


def kernel(in_features, token_positions, Wq, Wk, Wv, Wo):
    if "nc" not in _CACHE:
        _CACHE["nc"] = _build()
    nc = _CACHE["nc"]
    in_maps = _host_inputs(in_features, token_positions, Wq, Wk, Wv, Wo)
    res = run_bass_kernel_spmd(nc, in_maps, list(range(NC)))
    return _assemble(res.results)
